# revision 1
# baseline (speedup 1.0000x reference)
"""Trainium2 Bass kernel for nn_CrossAttention_28183575396415.

The reference block-mask gives every query exactly one key (kv = q_idx // 3),
so the softmax weight is identically 1 and the q/k projections, RMSNorm and
RoPE are dead code.  The module reduces to

    out[b, t] = x_kv[b, t // 3] @ Wv.T @ Wproj.T
              = x_kv[b, t // 3] @ WfT          with WfT = Wv.T @ Wproj.T

Strategy (8 NeuronCores, SPMD):
  - Host folds the two projection matrices into WfT (computed in float64,
    stored float32) — constant folding of adjacent linear layers.
  - The 4*2048 = 8192 kv rows are row-sharded 8 ways (1024 rows/core).
    Each core's shard is pre-transposed on host so every device DMA is a
    natural contiguous load and the PE needs no on-device transposes; the
    shard and the weight are concatenated into one [1024(k), 2048] input so
    each k-tile arrives in a single 1 MiB DMA:
        xw[:, :1024]  = x_shard.T   (k on partitions = contraction dim)
        xw[:, 1024:]  = WfT
  - Device: z = xT.T @ WfT with K accumulated in PSUM (8 k-tiles), then each
    z row tile is written to HBM three times (the t//3 replication), giving
    this core's contiguous [3072, 1024] slice of the flattened output.
  - Host unshard = concatenate the 8 slices.
"""

import json
import os

import numpy as np

import concourse.bass as bass
import concourse.mybir as mybir
from concourse.tile import TileContext
from concourse.vector_clock import ScopedClock
from concourse.bass_utils import run_bass_kernel_spmd

P = 128          # partitions
C = 1024         # model dim
K_T = C // P     # k tiles
M_T = C // P     # row tiles per core shard
N = 512          # matmul free dim (one PSUM bank of fp32)
L = 3            # replication factor (Tq // Tkv)
ROWS_PER_CORE = 1024
N_CORES = 8

# compute dtype: "f32r" (full-rate fp32 PE mode), "bf16", or "f32" (4x slower)
COMPUTE_DT = os.environ.get("KERNEL_COMPUTE_DT", "f32r")
# "device3": device writes the replicated [3072, 1024] slice (default)
# "host1":   device writes [1024, 1024]; host repeats rows (debug/compare only)
OUT_MODE = os.environ.get("KERNEL_OUT_MODE", "device3")


class SlimTailTileContext(TileContext):
    """Tile's kernel tail is drain -> barrier -> ~280 serialized per-semaphore
    clear instructions -> barrier (~8 us measured).  The clears only matter if
    the loaded NEFF executes more than once; every kernel() call here builds a
    fresh jit executable (fresh NEFF load, semaphores re-initialized), so skip
    them and the second barrier.  The drain still waits for every DMA queue,
    so outputs are complete before the program ends."""

    def _drain_and_barrier(self, tick_clock, wait_clock):
        # The SP drain (with its hoisted wait chain) already gates on every
        # engine's clock and every DMA queue, so outputs are complete when SP
        # retires; with no sem-clears to order, the closing all-engine
        # barrier adds nothing but latency.
        drain_inst = self.nc.sync.drain()
        wait_clock.add_sem_waits(
            drain_inst.ins, ScopedClock({None: tick_clock.global_clock})
        )
        popped = self.nc._tile_sem_poison_stack.pop()
        assert popped is self._sem_poison


def _split_multiwaits(nc: bass.Bass) -> None:
    """This container's walrus allows only ONE sync-wait on several
    instruction formats (Drain/CTRL, Matmult's LDWEIGHTS half, ...).  Tile
    can emit more.  Post-pass the serialized BIR: for any instruction with
    >1 on_wait, hoist all but the last wait onto single-wait EventSemaphore
    carriers inserted immediately before it on the same engine (waits then
    execute in queue order — semantics unchanged).  The patched JSON is
    pinned on the instance so every downstream serialization sees it."""
    raw = bass.Bass.to_json_bytes(nc)
    j = json.loads(raw)
    n_hoisted = 0
    for f in j["functions"]:
        for bb in f["blocks"]:
            new_insts = []
            for ins in bb["instructions"]:
                si = ins.get("sync_info")
                waits = si.get("on_wait", []) if si else []
                if len(waits) > 1:
                    for i, w in enumerate(waits[:-1]):
                        carrier = {
                            "engine": ins["engine"],
                            "ins": [],
                            "outs": [],
                            "name": f"{ins['name']}_hw{i}",
                            "opcode": "EventSemaphore",
                            "sync_info": {"on_update": [], "on_wait": [w]},
                        }
                        if "debug" in ins:
                            carrier["debug"] = ins["debug"]
                        new_insts.append(carrier)
                        n_hoisted += 1
                    si["on_wait"] = waits[-1:]
                new_insts.append(ins)
            bb["instructions"] = new_insts
    patched = json.dumps(j).encode()
    nc.to_json_bytes = lambda: patched


def _build(compute_dt: str, out_mode: str) -> bass.Bass:
    nc = bass.Bass("TRN2")
    in_mydt = {
        "bf16": mybir.dt.bfloat16,
        "f32r": mybir.dt.float32r,
        "f32": mybir.dt.float32,
    }[compute_dt]

    W2 = ROWS_PER_CORE + C  # concatenated [x | w] free dim
    xw = nc.dram_tensor("xw", [C, W2], in_mydt, kind="ExternalInput")
    n_rep = L if out_mode == "device3" else 1
    out = nc.dram_tensor(
        "out", [n_rep * ROWS_PER_CORE, C], mybir.dt.float32, kind="ExternalOutput"
    )

    xw_t = xw.rearrange("(t p) m -> t p m", p=P)  # [8, 128, 2048]
    # out row (n_rep*g + r) <- z row g
    out_rep = out.rearrange("(g r) c -> g r c", r=n_rep)  # [1024, n_rep, 1024]

    with SlimTailTileContext(nc) as tc:
        with (
            tc.tile_pool(name="xw", bufs=1) as xw_pool,
            tc.tile_pool(name="psum", bufs=8, space="PSUM") as psum_pool,
            tc.tile_pool(name="zout", bufs=6) as z_pool,
        ):
            # Load two k-tiles per DMA (2 MiB each, side by side in the free
            # dim) and alternate the trigger engine so the input stream isn't
            # paced by a single engine's ~1us-per-trigger issue cost.
            # First k-tile alone (1 MiB) so the PE can start as early as
            # possible; the rest in 2 MiB pair-DMAs to amortize trigger cost.
            in_eng = [nc.sync, nc.scalar]
            groups = [[0], [1, 2], [3, 4], [5, 6], [7]]
            xwk = [None] * K_T
            for j, grp in enumerate(groups):
                n = len(grp)
                t = xw_pool.tile([P, n * W2], in_mydt, name=f"xwp{j}", tag=f"xwp{j}")
                src = xw[grp[0] * P : (grp[0] + n) * P, :].rearrange(
                    "(g p) m -> p g m", p=P
                )
                dst = t[:].rearrange("p (g m) -> p g m", g=n)
                in_eng[j % 2].dma_start(dst, src)
                for i, k in enumerate(grp):
                    xwk[k] = (t, i * W2)

            # Two passes over the output-column halves.  Each pass keeps one
            # PSUM bank per row-tile (8 banks), accumulates over k in lockstep
            # with the input DMA stream, and its evictions/stores start right
            # after the last input byte — so the output DMA stream begins as
            # early as the data dependency allows and the two passes keep the
            # DMA engines saturated end-to-end.
            evict_eng = [
                lambda dst, src: nc.vector.tensor_copy(dst, src),
                lambda dst, src: nc.vector.tensor_copy(dst, src),
            ]
            out_eng = [nc.sync, nc.scalar]
            for cc in range(2):
                ps = [
                    psum_pool.tile([P, N], mybir.dt.float32, name=f"ps{cc}_{m}", tag="ps")
                    for m in range(M_T)
                ]
                for k in range(K_T):
                    tile_k, off = xwk[k]
                    rhs = tile_k[
                        :, off + ROWS_PER_CORE + cc * N : off + ROWS_PER_CORE + (cc + 1) * N
                    ]
                    for m in range(M_T):
                        nc.tensor.matmul(
                            ps[m][:],
                            tile_k[:, off + m * P : off + (m + 1) * P],
                            rhs,
                            start=(k == 0),
                            stop=(k == K_T - 1),
                        )
                for m in range(M_T):
                    zh = z_pool.tile([P, N], mybir.dt.float32, name=f"z{cc}_{m}", tag="z")
                    evict_eng[m % 2](zh[:], ps[m][:])
                    for r in range(n_rep):
                        out_eng[(m * n_rep + r) % 2].dma_start(
                            out_rep[m * P : (m + 1) * P, r, cc * N : (cc + 1) * N],
                            zh[:],
                        )

    _split_multiwaits(nc)
    return nc


_NC_CACHE: dict = {}


def _get_nc(compute_dt: str, out_mode: str) -> bass.Bass:
    key = (compute_dt, out_mode)
    if key not in _NC_CACHE:
        _NC_CACHE[key] = _build(compute_dt, out_mode)
    return _NC_CACHE[key]


def kernel(x_q, x_kv, Wq, Wk, Wv, Wproj, _compute_dt=None, _out_mode=None):
    compute_dt = _compute_dt or COMPUTE_DT
    out_mode = _out_mode or OUT_MODE
    B, Tkv, C_ = x_kv.shape
    assert (B, Tkv, C_) == (4, 2048, C)

    # Fold the two projections: z = x @ Wv.T @ Wproj.T = x @ WfT
    WfT = (Wv.astype(np.float64).T @ Wproj.astype(np.float64).T).astype(np.float32)

    x_flat = x_kv.reshape(B * Tkv, C)
    in_maps = []
    for c in range(N_CORES):
        shard = x_flat[c * ROWS_PER_CORE : (c + 1) * ROWS_PER_CORE]
        xw = np.concatenate([shard.T, WfT], axis=1)  # [C(k), 2048]
        if compute_dt == "bf16":
            import ml_dtypes

            xw = xw.astype(ml_dtypes.bfloat16)
        else:
            xw = np.ascontiguousarray(xw)
        in_maps.append({"xw": xw})

    nc = _get_nc(compute_dt, out_mode)
    res = run_bass_kernel_spmd(nc, in_maps, core_ids=list(range(N_CORES)))

    Tq = L * Tkv
    blocks = []
    for c in range(N_CORES):
        blk = res.results[c]["out"]
        if out_mode != "device3":
            blk = np.repeat(blk, L, axis=0)
        blocks.append(blk)
    out_flat = np.concatenate(blocks, axis=0)  # [B*Tq, C]
    return out_flat.reshape(B, Tq, C)



# revision 3
# speedup vs baseline: 1.3648x; 1.3648x over previous
"""Trainium2 Bass kernel for nn_CrossAttention_28183575396415.

The reference block-mask gives every query exactly one key (kv = q_idx // 3),
so the softmax weight is identically 1 and the q/k projections, RMSNorm and
RoPE are dead code.  The module reduces to

    out[b, t] = x_kv[b, t // 3] @ Wv.T @ Wproj.T
              = x_kv[b, t // 3] @ WfT          with WfT = Wv.T @ Wproj.T

Strategy (8 NeuronCores, SPMD):
  - Host folds the two projection matrices into WfT (float64 accumulate,
    stored bf16) and row-shards the 4*2048 = 8192 kv rows 8 ways (1024
    rows/core).  All device IO is bf16: 4 MiB in + 6 MiB out per core
    (vs 20 MiB for fp32), which moves the kernel from DMA-bound to the
    PE roofline.  rel_l2 error stays ~4e-3, far inside the 2e-2 gate.
  - Device pipeline: the 1024 shard rows are processed as 8 row-blocks of
    128.  Per block: 8 accumulating matmuls per PSUM column-half
    (lhsT = x.T k-tile, stationary; rhs = WfT k-tile, moving), eviction
    PSUM->SBUF with fp32->bf16 downcast, and 3 output DMAs (the t//3
    replication).  Dependencies are block-local, so input DMA, PE, evict
    and output DMA all stream concurrently -- no global barrier.
  - Host unshard = concatenate the 8 bf16 [3072, 1024] slices, upcast to
    fp32.
"""

import json

import numpy as np
import ml_dtypes

import concourse.bass as bass
import concourse.mybir as mybir
from concourse.tile import TileContext
from concourse.vector_clock import ScopedClock
from concourse.bass_utils import run_bass_kernel_spmd

P = 128          # partitions
C = 1024         # model dim
T_K = C // P     # contraction k-tiles
R_B = 8          # row blocks per core (1024 rows / 128)
N = 512          # matmul free dim (one PSUM bank of fp32)
L = 3            # replication factor (Tq // Tkv)
ROWS_PER_CORE = 1024
N_CORES = 8


class SlimTailTileContext(TileContext):
    """Tile's kernel tail is drain -> barrier -> ~280 serialized per-semaphore
    clear instructions -> barrier (~8 us measured).  The clears only matter if
    the loaded NEFF executes more than once; every kernel() call here builds a
    fresh jit executable (fresh NEFF load, semaphores re-initialized), so skip
    them and the second barrier.  The drain still waits for every DMA queue,
    so outputs are complete before the program ends."""

    def _drain_and_barrier(self, tick_clock, wait_clock):
        # The SP drain (with its hoisted wait chain) already gates on every
        # engine's clock and every DMA queue, so outputs are complete when SP
        # retires; with no sem-clears to order, the closing all-engine
        # barrier adds nothing but latency.
        drain_inst = self.nc.sync.drain()
        wait_clock.add_sem_waits(
            drain_inst.ins, ScopedClock({None: tick_clock.global_clock})
        )
        popped = self.nc._tile_sem_poison_stack.pop()
        assert popped is self._sem_poison

def _split_multiwaits(nc: bass.Bass) -> None:
    """This container's walrus allows only ONE sync-wait on several
    instruction formats (Drain/CTRL, Matmult's LDWEIGHTS half, ...).  Tile
    can emit more.  Post-pass the serialized BIR: for any instruction with
    >1 on_wait, hoist all but the last wait onto single-wait EventSemaphore
    carriers inserted immediately before it on the same engine (waits then
    execute in queue order — semantics unchanged).  The patched JSON is
    pinned on the instance so every downstream serialization sees it."""
    raw = bass.Bass.to_json_bytes(nc)
    j = json.loads(raw)
    n_hoisted = 0
    for f in j["functions"]:
        for bb in f["blocks"]:
            new_insts = []
            for ins in bb["instructions"]:
                si = ins.get("sync_info")
                waits = si.get("on_wait", []) if si else []
                if len(waits) > 1:
                    for i, w in enumerate(waits[:-1]):
                        carrier = {
                            "engine": ins["engine"],
                            "ins": [],
                            "outs": [],
                            "name": f"{ins['name']}_hw{i}",
                            "opcode": "EventSemaphore",
                            "sync_info": {"on_update": [], "on_wait": [w]},
                        }
                        if "debug" in ins:
                            carrier["debug"] = ins["debug"]
                        new_insts.append(carrier)
                        n_hoisted += 1
                    si["on_wait"] = waits[-1:]
                new_insts.append(ins)
            bb["instructions"] = new_insts
    patched = json.dumps(j).encode()
    nc.to_json_bytes = lambda: patched


def _build() -> bass.Bass:
    nc = bass.Bass("TRN2")
    bf16 = mybir.dt.bfloat16

    # xb[r*128 + kp, t*128 + row] = x_shard[r*128 + row, t*128 + kp]
    # i.e. per row-block r, the 8 stationary k-tiles side by side.
    xb = nc.dram_tensor("xb", [ROWS_PER_CORE, C], bf16, kind="ExternalInput")
    # wd = WfT  [k, c]
    wd = nc.dram_tensor("wd", [C, C], bf16, kind="ExternalInput")
    out = nc.dram_tensor(
        "out", [L * ROWS_PER_CORE, C], bf16, kind="ExternalOutput"
    )
    # out row (3*g + rep) <- z row g
    out_rep = out.rearrange("(g r) c -> g r c", r=L)  # [1024, 3, 1024]

    with SlimTailTileContext(nc) as tc:
        with (
            tc.tile_pool(name="xbp", bufs=1) as xb_pool,
            tc.tile_pool(name="wp", bufs=1) as w_pool,
            tc.tile_pool(name="psum", bufs=8, space="PSUM") as psum_pool,
            tc.tile_pool(name="zout", bufs=4) as z_pool,
        ):
            # Input streams: W k-tiles on SP (sync), x row-blocks on ACT
            # (scalar).  Separate DGE queues generate descriptors in
            # parallel; per-tile DMAs give the PE fine-grained pacing (a
            # matmul only waits for the one 0.25 MiB tile it reads, not a
            # whole multi-tile transfer).
            wt = []
            for t in range(T_K):
                w = w_pool.tile([P, C], bf16, name=f"w{t}", tag=f"w{t}")
                nc.sync.dma_start(w[:], wd[t * P : (t + 1) * P, :])
                wt.append(w)
            xbt = []
            for r in range(R_B):
                xr = xb_pool.tile([P, C], bf16, name=f"xb{r}", tag=f"xb{r}")
                nc.scalar.dma_start(xr[:], xb[r * P : (r + 1) * P, :])
                xbt.append(xr)

            out_eng = [nc.sync, nc.scalar, nc.gpsimd]
            for r in range(R_B):
                ps = [
                    psum_pool.tile([P, N], mybir.dt.float32, name=f"ps{r}_{h}", tag="ps")
                    for h in range(2)
                ]
                for t in range(T_K):
                    lhsT = xbt[r][:, t * P : (t + 1) * P]
                    for h in range(2):
                        nc.tensor.matmul(
                            ps[h][:],
                            lhsT,
                            wt[t][:, h * N : (h + 1) * N],
                            start=(t == 0),
                            stop=(t == T_K - 1),
                        )
                z = z_pool.tile([P, C], bf16, name=f"z{r}", tag="z")
                for h in range(2):
                    nc.vector.tensor_copy(z[:, h * N : (h + 1) * N], ps[h][:])
                for rep in range(L):
                    out_eng[(r * L + rep) % 3].dma_start(
                        out_rep[r * P : (r + 1) * P, rep, :], z[:]
                    )

    _split_multiwaits(nc)
    return nc


_NC_CACHE: dict = {}


def _get_nc() -> bass.Bass:
    if "nc" not in _NC_CACHE:
        _NC_CACHE["nc"] = _build()
    return _NC_CACHE["nc"]


def kernel(x_q, x_kv, Wq, Wk, Wv, Wproj):
    B, Tkv, C_ = x_kv.shape
    assert (B, Tkv, C_) == (4, 2048, C)

    # Fold the two projections: z = x @ Wv.T @ Wproj.T = x @ WfT
    WfT = (Wv.astype(np.float64).T @ Wproj.astype(np.float64).T).astype(
        ml_dtypes.bfloat16
    )

    x_flat = x_kv.reshape(B * Tkv, C).astype(ml_dtypes.bfloat16)
    in_maps = []
    for c in range(N_CORES):
        shard = x_flat[c * ROWS_PER_CORE : (c + 1) * ROWS_PER_CORE]
        # [r, row, t, kp] -> [r, kp, t, row]: per row-block, k on partitions
        xb = np.ascontiguousarray(
            shard.reshape(R_B, P, T_K, P).transpose(0, 3, 2, 1)
        ).reshape(ROWS_PER_CORE, C)
        in_maps.append({"xb": xb, "wd": WfT})

    nc = _get_nc()
    res = run_bass_kernel_spmd(nc, in_maps, core_ids=list(range(N_CORES)))

    Tq = L * Tkv
    out_flat = np.concatenate(
        [res.results[c]["out"].astype(np.float32) for c in range(N_CORES)], axis=0
    )  # [B*Tq, C]
    return out_flat.reshape(B, Tq, C)
